# revision 1
# baseline (speedup 1.0000x reference)
"""AttentionPooling Trainium2 kernel (8 NeuronCores, Bass/Tile).

Sharding: (batch, head-group) — core c handles batch b=c//2 and heads
4*(c%2)..4*(c%2)+3. Each core computes, for its 4 heads, Q^T/K^T (head-dim
major) and V (token major) projections, then a one-pass pooled attention:

  For each query stripe of 128 rows:  S = Q_stripe K^T / sqrt(d)  (PE, bf16)
  E = exp(S) (ScalarE), Z = rowsum(E) (VectorE), r = 1/Z (VectorE)
  w += r^T E (PE)   -- w[k] = sum_q E[q,k]/Z_q, PSUM-accumulated

  attended_mean * N = w @ V  (per head), then
  pooled_partial = concat_h(attended) @ (Wo_slice^T / N)

The mean-pool is folded through the output projection (linear), so the
(B,N,HID) attention output and the attn@V matmul are never materialized.
The V bias and output bias are folded on the host:
  pooled = pooled_partial(core even) + pooled_partial(core odd) + Wo@bv + bo

Pipelining: only head 0's Q/K projection runs as a prologue; the remaining
heads' Q/K and all V-projection matmuls are interleaved between attention
stripes so the TensorEngine fills the slack of the ScalarE-bound softmax.
PSUM budget (8 banks): S-stripe halves 2x[128,1024]f32 (4) + w accumulator
[128,1024]f32 (2, four (bank, partition-offset) sub-regions via matmul
column tile_position) + projection chunks 2x[128,512]f32 (2).

The host pre-transposes/casts the per-core operands (x[b]^T, W^T slices) so
the device does no transposes on the critical path; inputs are cast to bf16
(matmuls run at full PE rate; accumulation is fp32 in PSUM).
"""

import sys

import numpy as np

for _p in ("/opt/trn_rl_repo",):
    if _p not in sys.path:
        sys.path.append(_p)

import ml_dtypes

B, N, HID = 4, 2048, 1024
HEADS, HD = 8, 128
NH = 4          # heads per core
HGW = NH * HD   # head-group width (512)
NCORES = 8
P = 128
IT = HID // P   # 8 i-tiles
QT_TILES = N // P    # 16 query stripes
TOK_TILES = N // P   # 16 token tiles

BF16 = ml_dtypes.bfloat16

_cache = {}


def _build_nc():
    import concourse.bacc as bacc
    import concourse.tile as tile
    from concourse import mybir
    from concourse.bass import ds, ts
    from concourse.masks import make_identity
    from concourse.tile import add_dep_helper

    BF = mybir.dt.bfloat16
    F32 = mybir.dt.float32
    AF = mybir.ActivationFunctionType
    AX = mybir.AxisListType

    nc = bacc.Bacc(trn_type="TRN2")

    xT_d = nc.dram_tensor("xT", (HID, N), BF, kind="ExternalInput").ap()
    wqT_d = nc.dram_tensor("wqT", (NH, HID, HD), BF, kind="ExternalInput").ap()
    wkT_d = nc.dram_tensor("wkT", (NH, HID, HD), BF, kind="ExternalInput").ap()
    wvT_d = nc.dram_tensor("wvT", (HID, HGW), BF, kind="ExternalInput").ap()
    woT_d = nc.dram_tensor("woT", (HGW, HID), BF, kind="ExternalInput").ap()
    bq_d = nc.dram_tensor("bq_col", (P, NH), F32, kind="ExternalInput").ap()
    bk_d = nc.dram_tensor("bk_col", (P, NH), F32, kind="ExternalInput").ap()
    out_d = nc.dram_tensor("out_pooled", (1, HID), F32, kind="ExternalOutput").ap()

    inv_sqrt_d = float(1.0 / np.sqrt(HD))

    with tile.TileContext(nc) as tc:
        with (
            tc.tile_pool(name="persist", bufs=1) as persist,
            tc.tile_pool(name="sp", bufs=2, space="PSUM") as sp,
            tc.tile_pool(name="wp", bufs=1, space="PSUM") as wp,
            tc.tile_pool(name="pp", bufs=2, space="PSUM") as pp,
            tc.tile_pool(name="ep", bufs=3) as ep,
            tc.tile_pool(name="zp", bufs=4) as zp,
        ):
            # DMA order is the prologue critical path: head 0's Q/K weights
            # and the first token chunk of x^T land first so the first
            # projection matmuls start ~6us in; V/Wo/bias loads drain later
            # under the attention window.
            xT_sb = persist.tile([P, IT, N], BF)
            wq_sb = persist.tile([P, IT, NH, HD], BF)
            wk_sb = persist.tile([P, IT, NH, HD], BF)
            wv_sb = persist.tile([P, IT, HGW], BF)
            xT_r = xT_d.rearrange("(t p) n -> p t n", p=P)
            wqT_r = wqT_d.rearrange("h (t p) d -> h p t d", p=P)
            wkT_r = wkT_d.rearrange("h (t p) d -> h p t d", p=P)
            nc.sync.dma_start(out=wk_sb[:, :, 0, :], in_=wkT_r[0])
            nc.sync.dma_start(out=wq_sb[:, :, 0, :], in_=wqT_r[0])
            nc.sync.dma_start(out=wq_sb[:, :, 1, :], in_=wqT_r[1])
            nc.sync.dma_start(out=wk_sb[:, :, 1, :], in_=wkT_r[1])
            # x^T in two 2MiB halves: fewer per-DMA overheads, and the first
            # half's projection matmuls run while the second half transfers
            nc.sync.dma_start(out=xT_sb[:, : IT // 2, :], in_=xT_r[:, : IT // 2, :])
            nc.sync.dma_start(out=xT_sb[:, IT // 2 :, :], in_=xT_r[:, IT // 2 :, :])
            nc.sync.dma_start(
                out=wv_sb, in_=wvT_d.rearrange("(t p) d -> p t d", p=P)
            )
            for h in range(2, NH):
                nc.sync.dma_start(out=wq_sb[:, :, h, :], in_=wqT_r[h])
                nc.sync.dma_start(out=wk_sb[:, :, h, :], in_=wkT_r[h])
            wo_sb = persist.tile([P, NH, HID], BF)
            nc.sync.dma_start(out=wo_sb, in_=woT_d.rearrange("(t p) o -> p t o", p=P))
            bq_sb = persist.tile([P, NH], F32)
            bk_sb = persist.tile([P, NH], F32)
            nc.sync.dma_start(out=bq_sb, in_=bq_d)
            nc.sync.dma_start(out=bk_sb, in_=bk_d)
            ident = persist.tile([NH, NH], F32)
            make_identity(nc, ident)
            # one-hot columns: oneh_sb[p, h, h'] = 1.0 iff h == h'
            oneh_sb = persist.tile([P, NH, NH], BF)
            nc.vector.memset(oneh_sb, 0.0)
            for h in range(NH):
                nc.vector.memset(oneh_sb[:, h, h : h + 1], 1.0)
            zs4_sb = persist.tile([P, NH], BF)
            nc.vector.memset(zs4_sb, 0.0)

            QT_sb = persist.tile([P, NH, N], BF)
            KT_sb = persist.tile([P, NH, N], BF)
            V_sb = persist.tile([P, TOK_TILES, HGW], BF)
            w4_sb = persist.tile([NH, N], F32)
            # wTz[p, t, h, h'] = w_h[t*128+p] iff h' == h else 0 (block-diag
            # zero padding so per-head matmuls can emit 4-partition outputs)
            wTz_sb = persist.tile([P, TOK_TILES, NH, NH], BF)
            nc.vector.memset(wTz_sb, 0.0)
            att4_sb = persist.tile([NH, P], F32)
            attT_sb = persist.tile([P, NH], BF)
            pooled_sb = persist.tile([1, HID], F32)

            # last stripe-score matmul; background matmuls order behind it
            order_anchor = [None]

            def qk_chunk(proj_i, h, c, step=None, pool=None, tag="proj"):
                """One 512-token Q^T/K^T projection chunk for head h.
                As a generator (step=True) it yields after each 4-matmul
                half so background work interleaves in fine grains."""
                wsb, bsb, dst = (
                    (wq_sb, bq_sb, QT_sb),
                    (wk_sb, bk_sb, KT_sb),
                )[proj_i]
                ps = (pool or pp).tile([P, 512], F32, tag=tag, name="ps_qk")
                for i in range(IT):
                    mm = nc.tensor.matmul(
                        ps,
                        lhsT=wsb[:, i, h, :],
                        rhs=xT_sb[:, i, ts(c, 512)],
                        start=(i == 0),
                        stop=(i == IT - 1),
                    )
                    if False:
                        # keep background matmuls behind the latest stripe's
                        # score matmuls in the PE stream (scheduling-only dep;
                        # the greedy scheduler would otherwise front-load them
                        # and starve the ScalarE softmax pipeline)
                        add_dep_helper(
                            mm.ins, order_anchor[0].ins, sync=False, reason="bg-after-S"
                        )
                    if step and i == 3:
                        yield
                nc.vector.tensor_copy(dst[:, h, ts(c, 512)], ps)
                # per-partition bias (in-place, stride-0 free-dim broadcast)
                nc.vector.tensor_tensor(
                    dst[:, h, ts(c, 512)],
                    dst[:, h, ts(c, 512)],
                    bsb[:, h : h + 1].to_broadcast((P, 512)),
                    mybir.AluOpType.add,
                )
                if step:
                    yield

            def v_chunk(t, step=None):
                """One 128-token V projection tile (all 4 heads)."""
                ps = pp.tile([P, HGW], F32, tag="proj", name="ps_v")
                for i in range(IT):
                    mm = nc.tensor.matmul(
                        ps,
                        lhsT=xT_sb[:, i, ts(t, P)],
                        rhs=wv_sb[:, i, :],
                        start=(i == 0),
                        stop=(i == IT - 1),
                    )
                    if False:
                        add_dep_helper(
                            mm.ins, order_anchor[0].ins, sync=False, reason="bg-after-S"
                        )
                    if step and i == 3:
                        yield
                nc.vector.tensor_copy(V_sb[:, t, :], ps)
                if step:
                    yield

            # ---------------- prologue: head 0's K + first Q chunk --------
            # Stripe 0 needs all of K^T(h0) but only the first 128 queries of
            # Q^T(h0); the remaining Q chunks lead the background queue. The
            # five chunks borrow slots from all three PSUM pools so none of
            # them serializes on another's evacuation.
            for c, (pool_, tag_) in zip(
                range(4), ((pp, "proj"), (pp, "proj"), (sp, "s"), (sp, "s"))
            ):
                for _ in qk_chunk(1, 0, c, pool=pool_, tag=tag_):
                    pass
            for _ in qk_chunk(0, 0, 0, pool=wp, tag="w"):
                pass

            # Background projection work: remaining heads' Q/K and all V
            # tiles, emitted a few matmuls per stripe between the attention
            # matmul groups (the PE fills ScalarE-bound softmax slack).
            bg_tasks = []
            for c in range(1, 4):
                bg_tasks.append(qk_chunk(0, 0, c, step=True))
            for h2 in range(1, NH):
                for c in range(4):
                    bg_tasks.append(qk_chunk(0, h2, c, step=True))
                    bg_tasks.append(qk_chunk(1, h2, c, step=True))
                for t in range(NH * (h2 - 1), NH * h2):
                    bg_tasks.append(v_chunk(t, step=True))
            for t in range(NH * (NH - 1), NH * NH):
                bg_tasks.append(v_chunk(t, step=True))
            bg_tasks.reverse()  # consumed LIFO-from-front via pop() below
            BG_STEPS = 2 * len(bg_tasks)  # each generator yields twice
            BG_SPREAD = 48  # finish all background work by stripe 48 of 64

            def bg_advance(si):
                lo = si * BG_STEPS // BG_SPREAD
                hi = min((si + 1) * BG_STEPS // BG_SPREAD, BG_STEPS)
                for _ in range(max(0, hi - lo)):
                    while bg_tasks:
                        try:
                            next(bg_tasks[-1])
                            break
                        except StopIteration:
                            bg_tasks.pop()

            # ---------------- pooled attention ----------------
            # w accumulator: [128, 1024] fp32 = 2 PSUM banks. k-chunk j lives
            # at free range ts(j//2, 512), partitions [32*(j%2), +4) (heads on
            # partitions +0..3), via matmul column tile_position. Zero-matmuls
            # open each sub-region's accumulation group so later matmuls can
            # all use start=False regardless of has_written clear granularity.
            w4_ps = wp.tile([P, 1024], F32, tag="w", name="w4_ps")

            def w_region(j):
                poff = 32 * (j % 2)
                out = w4_ps[poff : poff + NH, ts(j // 2, 512)]
                tp = (0, poff) if poff else None
                return out, tp

            for j in range(4):
                out, tp = w_region(j)
                nc.tensor.matmul(
                    out,
                    lhsT=zs4_sb,
                    rhs=xT_sb[:, 0, ts(0, 512)],
                    start=True,
                    stop=False,
                    tile_position=tp,
                    skip_group_check=True,
                )

            def emit_S(h, qi):
                """Both k-half score matmul groups for one query stripe."""
                tiles = []
                for kk in range(2):
                    s_ps = sp.tile([P, 1024], F32, tag="s", name="s_ps")
                    for kc in range(2):
                        mm = nc.tensor.matmul(
                            s_ps[:, ts(kc, 512)],
                            lhsT=QT_sb[:, h, ts(qi, P)],
                            rhs=KT_sb[:, h, ds(kk * 1024 + kc * 512, 512)],
                            start=True,
                            stop=True,
                        )
                    tiles.append(s_ps)
                order_anchor[0] = mm
                return tiles

            def emit_w(pend, last):
                pe_, prb = pend
                for j in range(4):
                    out, tp = w_region(j)
                    nc.tensor.matmul(
                        out,
                        lhsT=prb,
                        rhs=pe_[:, ts(j, 512)],
                        start=False,
                        stop=last,
                        tile_position=tp,
                        skip_group_check=True,
                    )

            # Software-pipelined stripe loop: iteration (h, qi) consumes the
            # S tiles emitted in the previous iteration, emits the NEXT
            # stripe's S-matmuls first (so the exp chain never queues behind
            # other PE work), then the previous stripe's w-matmuls and a slice
            # of background projection work.
            pend_s = emit_S(0, 0)
            pend_w = None
            for h in range(NH):
                for qi in range(QT_TILES):
                    e_t = ep.tile([P, N], BF, tag="e", name="e_t")
                    zs = []
                    for kk, s_ps in enumerate(pend_s):
                        z_t = zp.tile([P, 1], F32, tag=f"z{kk}", name="z_t")
                        nc.scalar.activation(
                            out=e_t[:, ts(kk, 1024)],
                            in_=s_ps,
                            func=AF.Exp,
                            scale=inv_sqrt_d,
                            accum_out=z_t,
                        )
                        zs.append(z_t)
                    nqi = h * QT_TILES + qi + 1
                    if nqi < NH * QT_TILES:
                        pend_s = emit_S(nqi // QT_TILES, nqi % QT_TILES)
                    r_t = zp.tile([P, 1], F32, tag="r", name="r_t")
                    nc.vector.tensor_add(r_t, zs[0], zs[1])
                    nc.vector.reciprocal(r_t, r_t)
                    # rb4 column h = r (bf16), other columns zero
                    rb4_t = zp.tile([P, NH], BF, tag="rb", name="rb4_t")
                    nc.vector.tensor_tensor(
                        rb4_t,
                        oneh_sb[:, h, :],
                        r_t.to_broadcast((P, NH)),
                        mybir.AluOpType.mult,
                    )
                    if pend_w is not None:
                        emit_w(pend_w, False)
                    pend_w = (e_t, rb4_t)
                    # interleaved background projection work
                    bg_advance(h * QT_TILES + qi)
            emit_w(pend_w, True)

            # ---------------- tail: attended + output projection ----------
            for j in range(4):
                out, _ = w_region(j)
                nc.vector.tensor_copy(w4_sb[:, ts(j, 512)], out)
            for t in range(TOK_TILES):
                tp_ps = sp.tile([P, NH], F32, tag="s", name="tp_ps")
                nc.tensor.transpose(tp_ps, w4_sb[:, ts(t, P)], ident)
                # scatter into the block-diagonal (stride NH+1) positions
                nc.vector.tensor_copy(
                    wTz_sb[:, t].rearrange("p a b -> p (a b)")[:, :: NH + 1],
                    tp_ps,
                )
            att4_ps = sp.tile([NH, P], F32, tag="s", name="att4_ps")
            for t in range(TOK_TILES):
                for h in range(NH):
                    nc.tensor.matmul(
                        att4_ps,
                        lhsT=wTz_sb[:, t, h, :],
                        rhs=V_sb[:, t, ts(h, HD)],
                        start=(t == 0 and h == 0),
                        stop=(t == TOK_TILES - 1 and h == NH - 1),
                    )
            nc.vector.tensor_copy(att4_sb, att4_ps)
            attT_ps = sp.tile([P, NH], F32, tag="s", name="attT_ps")
            nc.tensor.transpose(attT_ps, att4_sb, ident)
            nc.vector.tensor_copy(attT_sb, attT_ps)
            p_ps = sp.tile([1, HID], F32, tag="s", name="p_ps")
            for oc in range(2):
                for h in range(NH):
                    nc.tensor.matmul(
                        p_ps[:, ts(oc, 512)],
                        lhsT=attT_sb[:, h : h + 1],
                        rhs=wo_sb[:, h, ts(oc, 512)],
                        start=(h == 0),
                        stop=(h == NH - 1),
                    )
            nc.vector.tensor_copy(pooled_sb, p_ps)
            nc.sync.dma_start(out=out_d, in_=pooled_sb)

    nc.finalize()  # Bacc: event-sem pass packs multi-waits into legal encodings
    return nc


def _get_nc():
    if "nc" not in _cache:
        _cache["nc"] = _build_nc()
    return _cache["nc"]


def _host_prep(inputs):
    """Build the 8 per-core input maps (host-side shard + transpose + cast)."""
    x = np.asarray(inputs["chunk_embeddings"], np.float32)
    in_maps = []
    for c in range(NCORES):
        b, hg = c // 2, c % 2
        sl = slice(hg * HGW, (hg + 1) * HGW)
        in_maps.append(
            {
                "xT": np.ascontiguousarray(x[b].T).astype(BF16),
                "wqT": np.ascontiguousarray(
                    np.asarray(inputs["Wq"], np.float32)[sl, :]
                    .T.reshape(HID, NH, HD)
                    .transpose(1, 0, 2)
                ).astype(BF16),
                "wkT": np.ascontiguousarray(
                    np.asarray(inputs["Wk"], np.float32)[sl, :]
                    .T.reshape(HID, NH, HD)
                    .transpose(1, 0, 2)
                ).astype(BF16),
                "wvT": np.ascontiguousarray(
                    np.asarray(inputs["Wv"], np.float32)[sl, :].T
                ).astype(BF16),
                "woT": np.ascontiguousarray(
                    np.asarray(inputs["Wo"], np.float32)[:, sl].T / np.float32(N)
                ).astype(BF16),
                "bq_col": np.ascontiguousarray(
                    np.asarray(inputs["bq"], np.float32)[sl].reshape(NH, P).T
                ),
                "bk_col": np.ascontiguousarray(
                    np.asarray(inputs["bk"], np.float32)[sl].reshape(NH, P).T
                ),
            }
        )
    return in_maps


def _unshard(results, inputs):
    bo = np.asarray(inputs["bo"], np.float32)
    bv = np.asarray(inputs["bv"], np.float32)
    Wo = np.asarray(inputs["Wo"], np.float32)
    bv_wo = Wo @ bv  # exact fold of the V bias through the output projection
    out = np.zeros((B, HID), np.float32)
    for b in range(B):
        out[b] = (
            results[2 * b]["out_pooled"][0]
            + results[2 * b + 1]["out_pooled"][0]
            + bv_wo
            + bo
        )
    return out


def _reference_numpy(inputs):
    """Fallback for non-trivial attention masks (never hit for the spec'd
    all-ones mask): straight numpy port of the reference."""
    x = np.asarray(inputs["chunk_embeddings"], np.float32)
    mask = np.asarray(inputs["attention_mask"])
    b, n, hid = x.shape

    def proj(W, bias):
        y = x @ np.asarray(W, np.float32).T + np.asarray(bias, np.float32)
        return y.reshape(b, n, HEADS, HD).transpose(0, 2, 1, 3)

    Q = proj(inputs["Wq"], inputs["bq"])
    K = proj(inputs["Wk"], inputs["bk"])
    V = proj(inputs["Wv"], inputs["bv"])
    s = np.einsum("bhqd,bhkd->bhqk", Q, K) / np.float32(np.sqrt(HD))
    s = np.where(mask[:, None, None, :] == 0, np.float32(-1e9), s)
    s = s - s.max(axis=-1, keepdims=True)
    e = np.exp(s)
    a = e / e.sum(axis=-1, keepdims=True)
    att = np.einsum("bhqk,bhkd->bhqd", a, V)
    att = att.transpose(0, 2, 1, 3).reshape(b, n, hid)
    out = att @ np.asarray(inputs["Wo"], np.float32).T + np.asarray(
        inputs["bo"], np.float32
    )
    m = mask[:, :, None].astype(np.float32)
    return (out * m).sum(axis=1) / m.sum(axis=1)


def _run(inputs, trace=False):
    from concourse.bass_utils import run_bass_kernel_spmd

    nc = _get_nc()
    in_maps = _host_prep(inputs)
    res = run_bass_kernel_spmd(
        nc, in_maps, core_ids=list(range(NCORES)), trace=trace
    )
    _cache["last_result"] = res
    return _unshard(res.results, inputs)


def kernel(**inputs):
    mask = np.asarray(inputs["attention_mask"])
    if not np.all(mask == 1):
        return _reference_numpy(inputs)
    return _run(inputs, trace=False)


def kernel_traced(**inputs):
    """Like kernel() but with NTFF profiling; returns (out, exec_time_ns)."""
    out = _run(inputs, trace=True)
    return out, _cache["last_result"].exec_time_ns



# revision 9
# speedup vs baseline: 1.0785x; 1.0785x over previous
"""AttentionPooling Trainium2 kernel (8 NeuronCores, Bass/Tile).

Sharding: (batch, head-group) — core c handles batch b=c//2 and heads
4*(c%2)..4*(c%2)+3. Each core computes, for its 4 heads, Q^T/K^T (head-dim
major) projections and V (token major), then a one-pass pooled attention:

  For each query stripe of 128 rows:  S = Q_stripe K^T  (PE, bf16)
  E = exp(S/(1024*sqrt(d))) (ScalarE, accum_out -> Z), r = 1/Z (VectorE)
  wacc += onehot(r)^T E  (PE, per-head PSUM accumulator [4,512], row j
  holding w[m*512+j*128+c] so each [4,128] block transposes to k-tiles)

  attended_h = sum_t wT[:,t] . V[t]  (PE, 1-col stationary)
  pooled = concat_h(attended) @ Wo_slice^T / N   (folded mean-pool)

Numerics: Q/K projections run fp8(e4m3)xfp8 in DoubleRow mode (2 MACs/PE
cell, contraction pairs (2v,2v+1) of 128) with weights pre-scaled by 32 on
the host (1/1024 folded into the exp scale); host-verified max rel err
5.6e-3 vs the fp32 reference. V projection stays bf16 (fp8 Wv error does
NOT average out through w@V). The K bias is dropped entirely: it only adds
a per-query constant to the scores, which softmax cancels. V/output biases
fold on the host: pooled += Wo@bv + bo.

Schedule: a dummy exp preloads the ACT table and ~3us of junk matmuls warm
the PE HAM clock-gate while the DMAs stream in (fp8 x first, 128x4KB
descriptors per 512-token chunk). Only K(h0)+Q(h0,c0) projections run as a
prologue; everything else (remaining Q/K chunks fp8-DR, V tiles bf16) is
interleaved between attention stripes by a cost/deadline-paced background
queue. Per-head w finalizes (transpose to wT) while the next head's
stripes run; attended matmuls run once V is resident; only head 3's
finalize + the 8 pooled-projection matmuls trail the last stripe.

PSUM (8 banks): S stripes 2x[128,1024] (4) + projection chunks 2x[128,512]
(2, also lent to the small finalize tiles) + per-head w accumulators
2x[4..16,512] (2).
"""

import math
import sys

import numpy as np

for _p in ("/opt/trn_rl_repo",):
    if _p not in sys.path:
        sys.path.append(_p)

import ml_dtypes

B, N, HID = 4, 2048, 1024
HEADS, HD = 8, 128
NH = 4          # heads per core
HGW = NH * HD   # head-group width (512)
NCORES = 8
P = 128
QT_TILES = N // P    # 16 query stripes per head
TOK_TILES = N // P   # 16 token tiles
NCHUNK = 4           # 512-token projection chunks

BF16 = ml_dtypes.bfloat16
F8 = ml_dtypes.float8_e4m3  # TRN fp8e4: max 240

_cache = {}


def _build_nc():
    import concourse.bacc as bacc
    import concourse.tile as tile
    from concourse import mybir
    from concourse.bass import ds, ts
    from concourse.masks import make_identity

    BF = mybir.dt.bfloat16
    F32 = mybir.dt.float32
    FP8 = mybir.dt.float8e4
    AF = mybir.ActivationFunctionType
    DR = mybir.MatmulPerfMode.DoubleRow

    nc = bacc.Bacc(trn_type="TRN2")

    # fp8 x, token-chunk major: xq8[pi, c, po, n'] = x[b, c*512+n', po*128+pi]
    xq8_d = nc.dram_tensor("xq8", (P, NCHUNK, 8, 512), FP8, kind="ExternalInput").ap()
    # fp8 Q/K weights (x32): wq8[pi, h, po, d] = 32*Wq[hg*512+h*128+d, po*128+pi]
    wq8_d = nc.dram_tensor("wq8", (P, NH, 8, P), FP8, kind="ExternalInput").ap()
    wk8_d = nc.dram_tensor("wk8", (P, NH, 8, P), FP8, kind="ExternalInput").ap()
    # bf16 x, hid-tile major: xbf[pi, po, n] = x[b, n, po*128+pi]
    xbf_d = nc.dram_tensor("xbf", (P, 8, N), BF, kind="ExternalInput").ap()
    # bf16 V weights: wvb[pi, po, o] = Wv[hg*512+o, po*128+pi]
    wvb_d = nc.dram_tensor("wvb", (P, 8, HGW), BF, kind="ExternalInput").ap()
    # bf16 O weights: wob[pi, h, o] = Wo[o, hg*512+h*128+pi]
    wob_d = nc.dram_tensor("wob", (P, NH, HID), BF, kind="ExternalInput").ap()
    bq_d = nc.dram_tensor("bq32", (P, NH), F32, kind="ExternalInput").ap()
    out_d = nc.dram_tensor("out_pooled", (1, HID), F32, kind="ExternalOutput").ap()

    inv_exp = float(1.0 / (1024.0 * math.sqrt(HD)))
    inv_pool = float(1.0 / N)

    with tile.TileContext(nc) as tc:
        with (
            tc.tile_pool(name="persist", bufs=1) as persist,
            tc.tile_pool(name="sp", bufs=2, space="PSUM") as sp,
            tc.tile_pool(name="pp", bufs=2, space="PSUM") as pp,
            tc.tile_pool(name="wp", bufs=2, space="PSUM") as wp,
            tc.tile_pool(name="ep", bufs=3) as ep,
            tc.tile_pool(name="zp", bufs=4) as zp,
        ):
            # ---- small constants (DVE) --------------------------------
            # mask16 columns {0,5,10,15} are 1: slicing [:, 4j:4j+4] gives
            # the one-hot column j used to route r into wacc row j.
            mask16 = persist.tile([P, 4 * NH], BF)
            nc.vector.memset(mask16, 0.0)
            for j in range(4):
                nc.vector.memset(mask16[:, 5 * j : 5 * j + 1], 1.0)
            ident4 = persist.tile([4, 4], F32)
            make_identity(nc, ident4)

            QT_sb = persist.tile([P, NH, N], BF)
            KT_sb = persist.tile([P, NH, N], BF)
            V_sb = persist.tile([P, TOK_TILES, HGW], BF)
            wacc_sb = persist.tile([4, 512], F32)
            # wT4[pi, h, m, j, :] = transposed w for head h, k-tile 4m+j,
            # one-hot at slot h (rest zero) so the attended matmuls of all 4
            # heads share one [4,128] PSUM accumulator (row h each) without
            # any cross-partition moves.
            wT4_sb = persist.tile([P, NH, 4, 4, NH], BF)
            nc.vector.memset(wT4_sb, 0.0)
            att4_sb = persist.tile([4, P], F32)
            attT_sb = persist.tile([P, NH], BF)
            pooled_sb = persist.tile([1, HID], F32)

            # ---- ACT table preload + PE warmup (run under the DMAs) ---
            zdum = zp.tile([P, 16], BF, tag="zd", name="zdum")
            nc.scalar.activation(out=zdum, in_=mask16, func=AF.Exp)
            for _ in range(10):
                warm_ps = pp.tile([16, 512], F32, tag="proj", name="warm_ps")
                nc.tensor.matmul(
                    warm_ps, lhsT=mask16, rhs=KT_sb[:, 0, 0:512],
                    start=True, stop=True, skip_group_check=True,
                )

            # ---- DMAs, critical-path first ----------------------------
            xq8_sb = persist.tile([P, NCHUNK, 8, 512], FP8)
            wq8_sb = persist.tile([P, NH, 8, P], FP8)
            wk8_sb = persist.tile([P, NH, 8, P], FP8)
            xbf_sb = persist.tile([P, 8, N], BF)
            wvb_sb = persist.tile([P, 8, HGW], BF)
            wob_sb = persist.tile([P, NH, HID], BF)
            bq_sb = persist.tile([P, NH], F32)

            nc.sync.dma_start(out=wk8_sb[:, 0], in_=wk8_d[:, 0])
            nc.sync.dma_start(out=wq8_sb[:, 0], in_=wq8_d[:, 0])
            for c in range(NCHUNK):
                nc.sync.dma_start(out=xq8_sb[:, c], in_=xq8_d[:, c])
            nc.sync.dma_start(out=bq_sb, in_=bq_d)
            for h in range(1, NH):
                nc.sync.dma_start(out=wk8_sb[:, h], in_=wk8_d[:, h])
                nc.sync.dma_start(out=wq8_sb[:, h], in_=wq8_d[:, h])
            for half in range(2):
                nc.sync.dma_start(
                    out=xbf_sb[:, 4 * half : 4 * half + 4, :],
                    in_=xbf_d[:, 4 * half : 4 * half + 4, :],
                )
            nc.sync.dma_start(out=wvb_sb, in_=wvb_d)
            nc.sync.dma_start(out=wob_sb, in_=wob_d)

            # ---- projection emitters ----------------------------------
            def qk_chunk(proj_i, h, c, step=False):
                """512-token fp8 DoubleRow Q^T/K^T projection for head h."""
                wsb, dst = ((wq8_sb, QT_sb), (wk8_sb, KT_sb))[proj_i]
                ps = pp.tile([P, 512], F32, tag="proj", name="ps_qk")
                for v in range(4):
                    nc.tensor.matmul(
                        ps,
                        lhsT=wsb[:, h, 2 * v : 2 * v + 2, :],
                        rhs=xq8_sb[:, c, 2 * v : 2 * v + 2, :],
                        start=(v == 0),
                        stop=(v == 3),
                        perf_mode=DR,
                    )
                    if step and v == 1:
                        yield
                if proj_i == 0:
                    # Q bias (32*bq) folded into the psum->bf16 evacuation
                    nc.vector.tensor_tensor(
                        dst[:, h, ts(c, 512)],
                        ps,
                        bq_sb[:, h : h + 1].to_broadcast((P, 512)),
                        mybir.AluOpType.add,
                    )
                else:
                    nc.vector.tensor_copy(dst[:, h, ts(c, 512)], ps)
                if step:
                    yield

            def v_chunk(t, step=False):
                """128-token bf16 V projection tile (all 4 heads)."""
                ps = pp.tile([P, HGW], F32, tag="proj", name="ps_v")
                for i in range(8):
                    nc.tensor.matmul(
                        ps,
                        lhsT=xbf_sb[:, i, ts(t, P)],
                        rhs=wvb_sb[:, i, :],
                        start=(i == 0),
                        stop=(i == 7),
                    )
                    if step and i in (2, 5):
                        yield
                nc.vector.tensor_copy(V_sb[:, t, :], ps)
                if step:
                    yield

            # ---- prologue: K(h0) + Q(h0,c0) ---------------------------
            for c in range(NCHUNK):
                for _ in qk_chunk(1, 0, c):
                    pass
            for _ in qk_chunk(0, 0, 0):
                pass

            # ---- background queue: (generator, est_ns, deadline) ------
            bg = []
            for c in range(1, NCHUNK):
                bg.append((qk_chunk(0, 0, c, True), 1000.0, 4 * c - 2))
            for h in range(1, NH):
                for c in range(NCHUNK):
                    bg.append((qk_chunk(1, h, c, True), 1000.0, 16 * h - 3))
                for c in range(NCHUNK):
                    bg.append((qk_chunk(0, h, c, True), 1000.0, 16 * h + 4 * c - 2))
            for t in range(TOK_TILES):
                bg.append((v_chunk(t, True), 2000.0, 47))
            bg_total = sum(u[1] for u in bg)
            bg_state = {"i": 0, "spent": 0.0, "frac": 0.0}
            BG_SPREAD = 56  # finish all background work by stripe 56 of 64

            def bg_step():
                gen, cost, _ = bg[bg_state["i"]]
                try:
                    next(gen)
                    bg_state["frac"] += 1.0
                    bg_state["spent"] += cost / 3.0
                except StopIteration:
                    bg_state["spent"] = (
                        sum(u[1] for u in bg[: bg_state["i"] + 1])
                    )
                    bg_state["i"] += 1

            def bg_advance(si):
                while bg_state["i"] < len(bg) and bg[bg_state["i"]][2] <= si + 1:
                    bg_step()
                target = (si + 1) * bg_total / BG_SPREAD
                while bg_state["i"] < len(bg) and bg_state["spent"] < target:
                    bg_step()

            # ---- per-head finalize + attended -------------------------
            wacc_tiles = {}
            att4_tile = [None]

            def finalize(h):
                """wacc (PSUM) -> wT4_sb[:, :, :, h] transposed k-tiles."""
                wps = wacc_tiles.pop(h)
                nc.vector.tensor_scalar_mul(wacc_sb, wps, inv_pool)
                for m in range(4):
                    tp = pp.tile([P, 4], F32, tag="proj", name="tp_ps")
                    nc.tensor.transpose(tp, wacc_sb[:, ts(m, P)], ident4)
                    nc.vector.tensor_copy(wT4_sb[:, h, m, :, h], tp)

            def attend(h):
                """att4[h, :] += sum_t wT4[:, t, h-onehot]^T V[t, head h]."""
                if att4_tile[0] is None:
                    att4_tile[0] = wp.tile([4, P], F32, tag="w", name="att4_ps")
                aps = att4_tile[0]
                for t in range(TOK_TILES):
                    nc.tensor.matmul(
                        aps,
                        lhsT=wT4_sb[:, h, t // 4, t % 4, :],
                        rhs=V_sb[:, t, ts(h, HD)],
                        start=(h == 0 and t == 0),
                        stop=(h == NH - 1 and t == TOK_TILES - 1),
                        skip_group_check=True,
                    )

            # ---- pooled attention stripe loop -------------------------
            def emit_S(h, qi):
                tiles = []
                for kk in range(2):
                    s_ps = sp.tile([P, 1024], F32, tag="s", name="s_ps")
                    for kc in range(2):
                        nc.tensor.matmul(
                            s_ps[:, ts(kc, 512)],
                            lhsT=QT_sb[:, h, ts(qi, P)],
                            rhs=KT_sb[:, h, ds(kk * 1024 + kc * 512, 512)],
                            start=True,
                            stop=True,
                        )
                    tiles.append(s_ps)
                return tiles

            def emit_w(pend):
                e_t, rb16, h, first, last = pend
                if first:
                    wacc_tiles[h] = wp.tile([4, 512], F32, tag="w", name="wacc")
                wps = wacc_tiles[h]
                out_ap = wps.rearrange("h (m c) -> h m c", m=4)
                e_r = e_t.rearrange("p (m c) -> p m c", m=4)
                for j in range(4):
                    nc.tensor.matmul(
                        out_ap,
                        lhsT=rb16[:, 4 * j : 4 * j + 4],
                        rhs=e_r[:, :, ts(j, P)],
                        start=(first and j == 0),
                        stop=(last and j == 3),
                        skip_group_check=True,
                    )

            pend_s = emit_S(0, 0)
            pend_w = None
            for gi in range(NH * QT_TILES):
                h, qi = gi // QT_TILES, gi % QT_TILES
                e_t = ep.tile([P, N], BF, tag="e", name="e_t")
                zs = []
                for kk, s_ps in enumerate(pend_s):
                    z_t = zp.tile([P, 1], F32, tag=f"z{kk}", name="z_t")
                    nc.scalar.activation(
                        out=e_t[:, ts(kk, 1024)],
                        in_=s_ps,
                        func=AF.Exp,
                        scale=inv_exp,
                        accum_out=z_t,
                    )
                    zs.append(z_t)
                if gi + 1 < NH * QT_TILES:
                    pend_s = emit_S((gi + 1) // QT_TILES, (gi + 1) % QT_TILES)
                r_t = zp.tile([P, 1], F32, tag="r", name="r_t")
                nc.vector.tensor_add(r_t, zs[0], zs[1])
                nc.vector.reciprocal(r_t, r_t)
                rb16 = zp.tile([P, 4 * NH], BF, tag="rb", name="rb16")
                nc.vector.tensor_tensor(
                    rb16,
                    mask16,
                    r_t.to_broadcast((P, 4 * NH)),
                    mybir.AluOpType.mult,
                )
                if pend_w is not None:
                    emit_w(pend_w)
                    wh = pend_w[2]
                    if pend_w[4]:  # closed head wh's accumulator
                        finalize(wh)
                        if wh == 2:
                            attend(0)
                            attend(1)
                pend_w = (e_t, rb16, h, qi == 0, qi == QT_TILES - 1)
                bg_advance(gi)

            emit_w(pend_w)
            finalize(3)
            attend(2)
            attend(3)
            nc.vector.tensor_copy(att4_sb, att4_tile[0])
            attT_ps = pp.tile([P, 4], F32, tag="proj", name="attT_ps")
            nc.tensor.transpose(attT_ps, att4_sb, ident4)
            nc.vector.tensor_copy(attT_sb, attT_ps)

            # ---- pooled output projection -----------------------------
            for oc in range(2):
                p_ps = pp.tile([1, 512], F32, tag="proj", name="p_ps")
                for h in range(NH):
                    nc.tensor.matmul(
                        p_ps,
                        lhsT=attT_sb[:, h : h + 1],
                        rhs=wob_sb[:, h, ts(oc, 512)],
                        start=(h == 0),
                        stop=(h == NH - 1),
                    )
                nc.vector.tensor_copy(pooled_sb[:, ts(oc, 512)], p_ps)
            nc.sync.dma_start(out=out_d, in_=pooled_sb)

    nc.finalize()
    return nc


def _get_nc():
    if "nc" not in _cache:
        _cache["nc"] = _build_nc()
    return _cache["nc"]


def _f8(a):
    return np.clip(a, -240.0, 240.0).astype(F8)


def _host_prep(inputs):
    """Build the 8 per-core input maps (shard + transpose + quantize)."""
    x = np.asarray(inputs["chunk_embeddings"], np.float32)
    wq = np.asarray(inputs["Wq"], np.float32)
    wk = np.asarray(inputs["Wk"], np.float32)
    wv = np.asarray(inputs["Wv"], np.float32)
    wo = np.asarray(inputs["Wo"], np.float32)
    bq = np.asarray(inputs["bq"], np.float32)
    in_maps = []
    for c in range(NCORES):
        b, hg = c // 2, c % 2
        sl = slice(hg * HGW, (hg + 1) * HGW)
        xT = x[b].T  # (1024, 2048): [po*128+pi, n]
        # xq8[pi, c, po, n'] = x[b, c*512+n', po*128+pi]
        xq8 = _f8(
            np.ascontiguousarray(
                xT.reshape(8, P, NCHUNK, 512).transpose(1, 2, 0, 3)
            )
        )
        # w?8[pi, h, po, d] = 32*W[hg*512+h*128+d, po*128+pi]
        def w8(W):
            m = (32.0 * W[sl, :]).T.reshape(8, P, NH, P).transpose(1, 2, 0, 3)
            return _f8(np.ascontiguousarray(m))
        # xbf[pi, po, n]
        xbf = np.ascontiguousarray(xT.reshape(8, P, N).transpose(1, 0, 2)).astype(
            BF16
        )
        # wvb[pi, po, o] = Wv[hg*512+o, po*128+pi]
        wvb = np.ascontiguousarray(
            wv[sl, :].T.reshape(8, P, HGW).transpose(1, 0, 2)
        ).astype(BF16)
        # wob[pi, h, o] = Wo[o, hg*512+h*128+pi]
        wob = np.ascontiguousarray(
            wo[:, sl].T.reshape(NH, P, HID).transpose(1, 0, 2)
        ).astype(BF16)
        bq32 = np.ascontiguousarray((32.0 * bq[sl]).reshape(NH, P).T)
        in_maps.append(
            {
                "xq8": xq8,
                "wq8": w8(wq),
                "wk8": w8(wk),
                "xbf": xbf,
                "wvb": wvb,
                "wob": wob,
                "bq32": bq32,
            }
        )
    return in_maps


def _unshard(results, inputs):
    bo = np.asarray(inputs["bo"], np.float32)
    bv = np.asarray(inputs["bv"], np.float32)
    Wo = np.asarray(inputs["Wo"], np.float32)
    bv_wo = Wo @ bv  # exact fold of the V bias through the output projection
    out = np.zeros((B, HID), np.float32)
    for b in range(B):
        out[b] = (
            results[2 * b]["out_pooled"][0]
            + results[2 * b + 1]["out_pooled"][0]
            + bv_wo
            + bo
        )
    return out


def _reference_numpy(inputs):
    """Fallback for non-trivial attention masks (never hit for the spec'd
    all-ones mask): straight numpy port of the reference."""
    x = np.asarray(inputs["chunk_embeddings"], np.float32)
    mask = np.asarray(inputs["attention_mask"])
    b, n, hid = x.shape

    def proj(W, bias):
        y = x @ np.asarray(W, np.float32).T + np.asarray(bias, np.float32)
        return y.reshape(b, n, HEADS, HD).transpose(0, 2, 1, 3)

    Q = proj(inputs["Wq"], inputs["bq"])
    K = proj(inputs["Wk"], inputs["bk"])
    V = proj(inputs["Wv"], inputs["bv"])
    s = np.einsum("bhqd,bhkd->bhqk", Q, K) / np.float32(np.sqrt(HD))
    s = np.where(mask[:, None, None, :] == 0, np.float32(-1e9), s)
    s = s - s.max(axis=-1, keepdims=True)
    e = np.exp(s)
    a = e / e.sum(axis=-1, keepdims=True)
    att = np.einsum("bhqk,bhkd->bhqd", a, V)
    att = att.transpose(0, 2, 1, 3).reshape(b, n, hid)
    out = att @ np.asarray(inputs["Wo"], np.float32).T + np.asarray(
        inputs["bo"], np.float32
    )
    m = mask[:, :, None].astype(np.float32)
    return (out * m).sum(axis=1) / m.sum(axis=1)


def _run(inputs, trace=False):
    from concourse.bass_utils import run_bass_kernel_spmd

    nc = _get_nc()
    in_maps = _host_prep(inputs)
    res = run_bass_kernel_spmd(
        nc, in_maps, core_ids=list(range(NCORES)), trace=trace
    )
    _cache["last_result"] = res
    return _unshard(res.results, inputs)


def kernel(**inputs):
    mask = np.asarray(inputs["attention_mask"])
    if not np.all(mask == 1):
        return _reference_numpy(inputs)
    return _run(inputs, trace=False)


def kernel_traced(**inputs):
    """Like kernel() but with NTFF profiling; returns (out, exec_time_ns)."""
    out = _run(inputs, trace=True)
    return out, _cache["last_result"].exec_time_ns
